# revision 8
# baseline (speedup 1.0000x reference)
"""Trainium2 Bass kernel for nn_KernelActivation (k=2 patch permutation).

The reference op is a pure element permutation of x:(16,64,224,224) fp32:
  view x as (b, i, p, j, q, w) = (16, 32, 2, 112, 2, 224)
  out  is  (b, i, j, w, p, q) flattened back to (16, 64, 224, 224)
i.e. out[b, i, j, w, p, q] = x[b, i, p, j, q, w].

Sharding: batch dim across 8 cores (2 batch elements per core), fully local.

Per-core program (16 tiles = 2 batches x 8 groups of 4 i-values):
  - tile partition dim = (i4 in 4, J in 28) = 112; J indexes groups of
    8 consecutive h-rows (= 4 output j-rows) of channel pair i = 4g+i4
  - load  x rows h=8J..8J+7 for c=2i, 2i+1 into [112, (p,h8,w)=3584]
    -> DMA descriptors are 7168B contiguous (8 rows x 896B)
  - 4 strided DVE copies (one per output row j4) interleave
    (p, q, w) -> (w, p, q) within each partition
  - store [112, (j4,w,p,q)=3584]: per partition the 4 output rows
    j=4J..4J+3 of plane i are contiguous in DRAM -> 14336B descriptors
Loads issue on the Sync HWDGE ring, stores on the Scalar HWDGE ring;
all shuffles run on Vector so ACT only issues store DMAs.
"""

import os
import sys

import numpy as np

sys.path.insert(0, "/opt/trn_rl_repo")

import concourse.bass as bass
import concourse.bacc as bacc
import concourse.mybir as mybir
import concourse.tile as tile
from concourse.bass_utils import run_bass_kernel_spmd

N_CORES = 8
B, C, H, W = 16, 64, 224, 224
K = 2
BPC = B // N_CORES  # batches per core
I, J = C // K, H // K  # 32, 112
G = 8   # i-groups per batch (4 i-values each)
I4 = 4  # i-values per tile
JP = 28  # h-octet groups per tile (8 rows each)
FREE = K * 8 * W  # (p, h8, w) = 3584 elements per partition
assert FREE == 3584
PADF = FREE // K + 16  # tin free size padded past 1792 to block AP flat-merge

_nc_cache = {}


def _build_program():
    key = "nc"
    if key in _nc_cache:
        return _nc_cache[key]

    nc = bacc.Bacc("TRN2", target_bir_lowering=False, debug=False)
    X = nc.dram_tensor("x", [BPC, C, H, W], mybir.dt.float32, kind="ExternalInput").ap()
    O = nc.dram_tensor(
        "out", [BPC, C, H, W], mybir.dt.float32, kind="ExternalOutput"
    ).ap()

    # x viewed as (b, g, i4, J, p, h8, w): c = 8g + 2*i4 + p, h = 8J + h8
    Xv = X.rearrange(
        "b (g i4 p) (J h8) w -> b g i4 J p h8 w", g=G, i4=I4, p=K, J=JP, h8=8
    )
    # out flat per (b): (i, j, w, p, q) lexicographic == (c, h, w); group as
    # (g, i4, J, f) with f = (j4, w, p, q) = 3584 contiguous elements
    Ov = O.rearrange("b c h w -> b (c h w)").rearrange(
        "b (g i4 J f) -> b g (i4 J) f", g=G, i4=I4, J=JP, f=FREE
    )

    with tile.TileContext(nc) as tc:
        with (
            tc.tile_pool(name="tin", bufs=4) as tin_pool,
            tc.tile_pool(name="tout", bufs=4) as tout_pool,
        ):
            for b in range(BPC):
                for g in range(G):
                    # ---- load: the SBUF side of a DMA needs the partition
                    # dim first and affine, so issue one 28-partition DMA
                    # per (i4, p) into partition range [28*i4, 28*i4+28).
                    # Each descriptor is 8 h-rows = 7168B contiguous.
                    # tin tiles padded to 1808 so the AP balancer cannot
                    # flat-merge the partition dim into the free dim.
                    t_ins = []
                    for p in range(K):
                        t_in = tin_pool.tile([I4 * JP, PADF], mybir.dt.float32)
                        for i4 in range(I4):
                            nc.sync.dma_start(
                                out=t_in[28 * i4 : 28 * (i4 + 1), 0 : FREE // K],
                                in_=Xv[b, g, i4, :, p],
                            )
                        t_ins.append(t_in)

                    # ---- shuffle: 8 strided copies, free (w,q) <- (q,w)
                    # per (j4, p); DVE gets 5, ACT 3 (ACT ~2cyc/elem here)
                    t_out = tout_pool.tile([I4 * JP, FREE], mybir.dt.float32)
                    dstv = t_out.rearrange(
                        "P (j4 w p q) -> P j4 p w q", j4=4, w=W, p=K, q=K
                    )
                    cnt = 0
                    for j4 in range(4):
                        for p in range(K):
                            srcv = t_ins[p][:, 0 : FREE // K].rearrange(
                                "P (j4 q w) -> P j4 w q", j4=4, q=K, w=W
                            )
                            eng = nc.vector if cnt % 8 < 5 else nc.scalar
                            cnt += 1
                            if eng is nc.vector:
                                eng.tensor_copy(
                                    out=dstv[:, j4, p], in_=srcv[:, j4]
                                )
                            else:
                                eng.copy(out=dstv[:, j4, p], in_=srcv[:, j4])

                    # ---- store: per partition one contiguous 14336B run
                    nc.scalar.dma_start(out=Ov[b, g], in_=t_out[:])

    nc.compile()
    _nc_cache[key] = nc
    return nc


def kernel(x: np.ndarray) -> np.ndarray:
    x = np.ascontiguousarray(np.asarray(x, dtype=np.float32))
    assert x.shape == (B, C, H, W), x.shape

    nc = _build_program()
    in_maps = [{"x": x[c * BPC : (c + 1) * BPC]} for c in range(N_CORES)]
    trace = bool(int(os.environ.get("KERNEL_TRACE", "0")))
    res = run_bass_kernel_spmd(nc, in_maps, list(range(N_CORES)), trace=trace)
    if trace:
        _nc_cache["last_results"] = res
    out = np.concatenate([res.results[c]["out"] for c in range(N_CORES)], axis=0)
    return out


# revision 11
# speedup vs baseline: 1.1675x; 1.1675x over previous
"""Trainium2 Bass kernel for nn_KernelActivation (k=2 patch permutation).

The reference op is a pure element permutation of x:(16,64,224,224) fp32:
  view x as (b, i, p, j, q, w) = (16, 32, 2, 112, 2, 224)
  out  is  (b, i, j, w, p, q) flattened back to (16, 64, 224, 224)
i.e. out[b, i, j, w, p, q] = x[b, i, p, j, q, w].

Sharding: batch dim across 8 cores (2 batch elements per core), fully local.

Per-core program (16 tiles = 2 batches x 8 groups of 4 i-values):
  - tile partition dim = (i4 in 4, J in 28) = 112; J indexes groups of
    8 consecutive h-rows (= 4 output j-rows) of channel pair i = 4g+i4
  - load  x rows h=8J..8J+7 for c=2i, 2i+1 into [112, (p,h8,w)=3584]
    -> DMA descriptors are 7168B contiguous (8 rows x 896B)
  - 4 strided DVE copies (one per output row j4) interleave
    (p, q, w) -> (w, p, q) within each partition
  - store [112, (j4,w,p,q)=3584]: per partition the 4 output rows
    j=4J..4J+3 of plane i are contiguous in DRAM -> 14336B descriptors
Loads issue on the Sync HWDGE ring, stores on the Scalar HWDGE ring;
all shuffles run on Vector so ACT only issues store DMAs.
"""

import os
import sys

import numpy as np

sys.path.insert(0, "/opt/trn_rl_repo")

import concourse.bass as bass
import concourse.bacc as bacc
import concourse.mybir as mybir
import concourse.tile as tile
from concourse.bass_utils import run_bass_kernel_spmd

N_CORES = 8
B, C, H, W = 16, 64, 224, 224
K = 2
BPC = B // N_CORES  # batches per core
I, J = C // K, H // K  # 32, 112
G = 8   # i-groups per batch (4 i-values each)
I4 = 4  # i-values per tile
JP = 28  # h-octet groups per tile (8 rows each)
FREE = K * 8 * W  # (p, h8, w) = 3584 elements per partition
assert FREE == 3584
PADF = FREE // K + 16  # tin free size padded past 1792 to block AP flat-merge

_nc_cache = {}


def _build_program():
    key = "nc"
    if key in _nc_cache:
        return _nc_cache[key]

    nc = bacc.Bacc("TRN2", target_bir_lowering=False, debug=False)
    X = nc.dram_tensor("x", [BPC, C, H, W], mybir.dt.float32, kind="ExternalInput").ap()
    O = nc.dram_tensor(
        "out", [BPC, C, H, W], mybir.dt.float32, kind="ExternalOutput"
    ).ap()

    # x viewed as (b, g, i4, J, p, h8, w): c = 8g + 2*i4 + p, h = 8J + h8
    Xv = X.rearrange(
        "b (g i4 p) (J h8) w -> b g i4 J p h8 w", g=G, i4=I4, p=K, J=JP, h8=8
    )
    # out flat per (b): (i, j, w, p, q) lexicographic == (c, h, w); group as
    # (g, i4, J, f) with f = (j4, w, p, q) = 3584 contiguous elements
    Ov = O.rearrange("b c h w -> b (c h w)").rearrange(
        "b (g i4 J f) -> b g i4 J f", g=G, i4=I4, J=JP, f=FREE
    )

    with tile.TileContext(nc) as tc:
        with (
            tc.tile_pool(name="tin", bufs=4) as tin_pool,
            tc.tile_pool(name="tout", bufs=4) as tout_pool,
        ):
            for b in range(BPC):
                for g in range(G):
                    # ---- load: the SBUF side of a DMA needs the partition
                    # dim first and affine, so issue one 28-partition DMA
                    # per (i4, p). Physical partition P = 4*J + i4: the
                    # stride-4 interleave makes every DMA span all 8 SBUF
                    # clusters and both port halves = all 16 AXI ports
                    # (a contiguous 28-range would hit only ~7 ports).
                    # Each descriptor is 8 h-rows = 7168B contiguous.
                    # tin tiles padded to 1808 so the AP balancer cannot
                    # flat-merge the partition dim into the free dim.
                    t_ins = []
                    for p in range(K):
                        t_in = tin_pool.tile([I4 * JP, PADF], mybir.dt.float32)
                        tv = t_in.rearrange("(J i4) f -> i4 J f", J=JP, i4=I4)
                        for i4 in range(I4):
                            nc.sync.dma_start(
                                out=tv[i4][:, 0 : FREE // K],
                                in_=Xv[b, g, i4, :, p],
                            )
                        t_ins.append(t_in)

                    # ---- shuffle: 8 strided copies, free (w,q) <- (q,w)
                    # per (j4, p); DVE gets 5, ACT 3 (ACT ~2cyc/elem here)
                    t_out = tout_pool.tile([I4 * JP, FREE], mybir.dt.float32)
                    dstv = t_out.rearrange(
                        "P (j4 w p q) -> P j4 p w q", j4=4, w=W, p=K, q=K
                    )
                    cnt = 0
                    for j4 in range(4):
                        for p in range(K):
                            srcv = t_ins[p][:, 0 : FREE // K].rearrange(
                                "P (j4 q w) -> P j4 w q", j4=4, q=K, w=W
                            )
                            eng = nc.vector if cnt % 8 < 5 else nc.scalar
                            cnt += 1
                            if eng is nc.vector:
                                eng.tensor_copy(
                                    out=dstv[:, j4, p], in_=srcv[:, j4]
                                )
                            else:
                                eng.copy(out=dstv[:, j4, p], in_=srcv[:, j4])

                    # ---- store: per partition one contiguous 14336B run;
                    # one DMA per i4 (28 stride-4 partitions, all 16 ports)
                    tov = t_out.rearrange("(J i4) f -> i4 J f", J=JP, i4=I4)
                    for i4 in range(I4):
                        nc.scalar.dma_start(out=Ov[b, g, i4], in_=tov[i4])

    nc.compile()
    _nc_cache[key] = nc
    return nc


def kernel(x: np.ndarray) -> np.ndarray:
    x = np.ascontiguousarray(np.asarray(x, dtype=np.float32))
    assert x.shape == (B, C, H, W), x.shape

    nc = _build_program()
    in_maps = [{"x": x[c * BPC : (c + 1) * BPC]} for c in range(N_CORES)]
    trace = bool(int(os.environ.get("KERNEL_TRACE", "0")))
    res = run_bass_kernel_spmd(nc, in_maps, list(range(N_CORES)), trace=trace)
    if trace:
        _nc_cache["last_results"] = res
    out = np.concatenate([res.results[c]["out"] for c in range(N_CORES)], axis=0)
    return out


# revision 12
# speedup vs baseline: 1.1742x; 1.0058x over previous
"""Trainium2 Bass kernel for nn_KernelActivation (k=2 patch permutation).

The reference op is a pure element permutation of x:(16,64,224,224) fp32:
  view x as (b, i, p, j, q, w) = (16, 32, 2, 112, 2, 224)
  out  is  (b, i, j, w, p, q) flattened back to (16, 64, 224, 224)
i.e. out[b, i, j, w, p, q] = x[b, i, p, j, q, w].

Sharding: batch dim across 8 cores (2 batch elements per core), fully local.

Per-core program: 16 tile-sets = 2 batches x 8 groups of 4 i-values.
Partition map P = 28*i4 + J (i4 in 4, J in 28 h-octets = 4 output rows).

  - load: 8 DMAs per set, one per (i4, p), each 28 partitions x 4
    strided 448-element runs -> 112 descriptors of 1792B per DMA
    (112 = 7 per SDMA engine engages all 16 engines; the SBUF free
    layout (j4, p, q, w) interleaves p so descriptors cannot merge).
    Loads alternate between the Sync and Scalar HWDGE rings so the
    port-poor 28-partition blocks of different i4 interleave.
  - shuffle: 8 DVE copies per set, free (w,q) <- (q,w) per (j4, p),
    [112 partitions x 448 elements] each.
  - store: ONE 112-partition DMA per set; per partition the 4 output
    rows j=4J..4J+3 of plane i are contiguous -> 112 x 14336B
    descriptors (measured 22.2 GB/s/engine, the best store shape).
"""

import os
import sys

import numpy as np

sys.path.insert(0, "/opt/trn_rl_repo")

import concourse.bass as bass
import concourse.bacc as bacc
import concourse.mybir as mybir
import concourse.tile as tile
from concourse.bass_utils import run_bass_kernel_spmd

N_CORES = 8
B, C, H, W = 16, 64, 224, 224
K = 2
BPC = B // N_CORES  # batches per core
G = 8   # i-groups per batch (4 i-values each)
I4 = 4  # i-values per tile-set
JP = 28  # h-octet groups (8 rows = 4 output rows) per plane
FREE = 4 * K * K * W  # 3584 elements per partition
PADF = FREE + 16  # tin free size padded so partition stride != extents

_nc_cache = {}


def _build_program():
    key = "nc"
    if key in _nc_cache:
        return _nc_cache[key]

    nc = bacc.Bacc("TRN2", target_bir_lowering=False, debug=False)
    X = nc.dram_tensor("x", [BPC, C, H, W], mybir.dt.float32, kind="ExternalInput").ap()
    O = nc.dram_tensor(
        "out", [BPC, C, H, W], mybir.dt.float32, kind="ExternalOutput"
    ).ap()

    # x viewed as (b, g, i4, p, J, j4, (q w)): c = 8g+2i4+p, h = 8J+2j4+q
    Xv = X.rearrange(
        "b (g i4 p) (J j4 q) w -> b g i4 p J j4 (q w)",
        g=G, i4=I4, p=K, J=JP, j4=4, q=K,
    )
    # out flat per b is (i, j, w, p, q) lexicographic; partition-major
    # (i4 J) with f = (j4, w, p, q) = 3584 contiguous elements
    Ov = O.rearrange("b c h w -> b (c h w)").rearrange(
        "b (g i4 J f) -> b g (i4 J) f", g=G, i4=I4, J=JP, f=FREE
    )

    with tile.TileContext(nc) as tc:
        with (
            tc.tile_pool(name="tin", bufs=6) as tin_pool,
            tc.tile_pool(name="tout", bufs=5) as tout_pool,
        ):
            n_set = 0
            for b in range(BPC):
                for g in range(G):
                    # ---- load: 8 DMAs, one per (i4, p)
                    t_in = tin_pool.tile([I4 * JP, PADF], mybir.dt.float32)
                    tv = t_in[:, 0:FREE].rearrange(
                        "P (j4 p f) -> P j4 p f", j4=4, p=K, f=K * W
                    )
                    for i4 in range(I4):
                        for p in range(K):
                            eng = nc.sync if (i4 + p) % 2 == 0 else nc.scalar
                            eng.dma_start(
                                out=tv[28 * i4 : 28 * (i4 + 1), :, p],
                                in_=Xv[b, g, i4, p],
                            )

                    # ---- shuffle: 8 DVE copies, (w, q) <- (q, w)
                    t_out = tout_pool.tile([I4 * JP, FREE], mybir.dt.float32)
                    src3 = t_in[:, 0:FREE].rearrange(
                        "P (j4 p q w) -> P j4 p w q", j4=4, p=K, q=K, w=W
                    )
                    dst3 = t_out.rearrange(
                        "P (j4 w p q) -> P j4 p w q", j4=4, w=W, p=K, q=K
                    )
                    for j4 in range(4):
                        for p in range(K):
                            nc.vector.tensor_copy(
                                out=dst3[:, j4, p], in_=src3[:, j4, p]
                            )

                    # ---- store: one 112-partition DMA, 14336B descriptors
                    eng = nc.scalar if n_set % 2 == 0 else nc.sync
                    eng.dma_start(out=Ov[b, g], in_=t_out[:])
                    n_set += 1

    nc.compile()
    _nc_cache[key] = nc
    return nc


def kernel(x: np.ndarray) -> np.ndarray:
    x = np.ascontiguousarray(np.asarray(x, dtype=np.float32))
    assert x.shape == (B, C, H, W), x.shape

    nc = _build_program()
    in_maps = [{"x": x[c * BPC : (c + 1) * BPC]} for c in range(N_CORES)]
    trace = bool(int(os.environ.get("KERNEL_TRACE", "0")))
    res = run_bass_kernel_spmd(nc, in_maps, list(range(N_CORES)), trace=trace)
    if trace:
        _nc_cache["last_results"] = res
    out = np.concatenate([res.results[c]["out"] for c in range(N_CORES)], axis=0)
    return out


# revision 13
# speedup vs baseline: 1.4889x; 1.2680x over previous
"""Trainium2 Bass kernel for nn_KernelActivation (k=2 patch permutation).

The reference op is a pure element permutation of x:(16,64,224,224) fp32:
  view x as (b, i, p, j, q, w) = (16, 32, 2, 112, 2, 224)
  out  is  (b, i, j, w, p, q) flattened back to (16, 64, 224, 224)
i.e. out[b, i, j, w, p, q] = x[b, i, p, j, q, w].

Sharding: batch dim across 8 cores (2 batch elements per core), fully local.

Partition map P = j (112 partitions) for every DMA: the only map that is
affine for loads AND store, uses all 16 SBUF AXI ports, and (with >=
224 descriptors / >=400KB per DMA) spreads descriptors over all 16 SDMA
engines (the DGE hands out chunks of max(ceil(n/16), 14336B/desc) descs
per engine starting at engine 0 -- small DMAs land on only 14 engines).

Per-core program: 32 quads = 2 batches x 8 groups of 4 i-values:
  - 4 loads (one per i): [112, (p,q,w)=896] <- x[b,i] ; one DMA of
    224 x 1792B descriptors (measured 21.7 GB/s/engine)
  - 4 DVE copies (one per i): free (w,p,q) <- (p,q,w), 4D-AP strided
  - 1 store for the quad: t_out [112, (i4,w,p,q)=3584] -> DRAM; one
    DMA of 448 x 3584B descriptors over all 16 engines
Loads issue on the Sync HWDGE ring, stores on the Scalar ring; all
shuffle copies run on Vector so ACT only issues store DMAs.
"""

import os
import sys

import numpy as np

sys.path.insert(0, "/opt/trn_rl_repo")

import concourse.bass as bass
import concourse.bacc as bacc
import concourse.mybir as mybir
import concourse.tile as tile
from concourse.bass_utils import run_bass_kernel_spmd

N_CORES = 8
B, C, H, W = 16, 64, 224, 224
K = 2
BPC = B // N_CORES  # batches per core
I, J = C // K, H // K  # 32, 112
G = 8   # quads per batch
I4 = 4  # i-values per quad
FREE1 = K * K * W      # 896 els per partition per i
FREE4 = I4 * FREE1     # 3584 els per partition per quad

_nc_cache = {}


def _build_program():
    key = "nc"
    if key in _nc_cache:
        return _nc_cache[key]

    nc = bacc.Bacc("TRN2", target_bir_lowering=False, debug=False)
    X = nc.dram_tensor("x", [BPC, C, H, W], mybir.dt.float32, kind="ExternalInput").ap()
    O = nc.dram_tensor(
        "out", [BPC, C, H, W], mybir.dt.float32, kind="ExternalOutput"
    ).ap()

    # x as (b, i, p, j, (q w)): c = 2i + p, h = 2j + q
    Xv = X.rearrange("b (i p) (j q) w -> b i j p (q w)", i=I, p=K, j=J, q=K)
    # out flat per b is (i, j, w, p, q) lexicographic; per quad g:
    # (j, i4, f) with f = (w p q) = 896 contiguous elements
    Ov = O.rearrange("b c h w -> b (c h w)").rearrange(
        "b (g i4 j f) -> b g j i4 f", g=G, i4=I4, j=J, f=FREE1
    )

    with tile.TileContext(nc) as tc:
        with (
            tc.tile_pool(name="tin", bufs=12) as tin_pool,
            tc.tile_pool(name="tout", bufs=4) as tout_pool,
        ):
            for b in range(BPC):
                for g in range(G):
                    t_out = tout_pool.tile([J, FREE4], mybir.dt.float32)
                    dstv = t_out.rearrange(
                        "j (i4 w p q) -> j i4 w p q", i4=I4, w=W, p=K, q=K
                    )
                    for i4 in range(I4):
                        i = g * I4 + i4
                        # ---- load: [j, (p, q, w)] ; 224 x 1792B descs
                        t_in = tin_pool.tile([J, FREE1], mybir.dt.float32)
                        nc.sync.dma_start(out=t_in[:], in_=Xv[b, i])

                        # ---- shuffle: free (w,p,q) <- (p,q,w) on DVE
                        srcv = t_in.rearrange(
                            "j (p q w) -> j w p q", p=K, q=K, w=W
                        )
                        nc.vector.tensor_copy(out=dstv[:, i4], in_=srcv)

                    # ---- store: one DMA per quad, 448 x 3584B descs
                    nc.scalar.dma_start(out=Ov[b, g], in_=t_out[:])

    nc.compile()
    _nc_cache[key] = nc
    return nc


def kernel(x: np.ndarray) -> np.ndarray:
    x = np.ascontiguousarray(np.asarray(x, dtype=np.float32))
    assert x.shape == (B, C, H, W), x.shape

    nc = _build_program()
    in_maps = [{"x": x[c * BPC : (c + 1) * BPC]} for c in range(N_CORES)]
    trace = bool(int(os.environ.get("KERNEL_TRACE", "0")))
    res = run_bass_kernel_spmd(nc, in_maps, list(range(N_CORES)), trace=trace)
    if trace:
        _nc_cache["last_results"] = res
    out = np.concatenate([res.results[c]["out"] for c in range(N_CORES)], axis=0)
    return out
